# revision 11
# baseline (speedup 1.0000x reference)
"""Trainium2 Bass kernel for windowed multi-head attention (ClassicAttention).

Shapes (hardcoded per spec): x (1024, 68, 768), pe (128, 768), mask zeros.
Data-parallel over 8 NeuronCores on the leading window axis.

v5 (v3 @1026us, v2 baseline @1719us; v4's xbar DMA-transpose regressed to
1825us - the DMA_TRANSPOSE instruction costs ~1.2us on the issuing sync
queue, starving the PE - so V transposes stay on the PE):
- v3: softmax 1/rowsum broadcast via an all-ones-stationary matmul (PSUM
  holds row sums replicated across partitions) instead of a
  partition-stride-0 broadcast DMA that flooded all 16 DMA queues; V
  computed weights-stationary then PE-transposed per (window, o-tile);
  contiguous per-group DRAM slabs; split weight loads.
- v5: groups of G=7 windows (18x7 + 1x2) - the 476-row moving operand is
  the largest that keeps a fp32 PSUM accumulator within one 2KB bank,
  cutting the 128x128-stationary matmul count from 4608 to 2052 and
  amortizing the ~27ns per-matmul overhead; proj drains on the Act
  engine with a per-partition bias AP so the next group's QK-gen is not
  gated on the Vector queue; window-major DRAM slabs so uneven groups
  stay contiguous.
"""

import os
import sys

for _p in (
    "/root/.axon_site",
    "/root/.axon_site/_ro/trn_rl_repo",
    "/root/.axon_site/_ro/pypackages",
    "/opt/trn_rl_repo",
):
    if os.path.isdir(_p) and _p not in sys.path:
        sys.path.append(_p)

import ml_dtypes
import numpy as np

import concourse.bass as bass
import concourse.mybir as mybir
import concourse.tile as tile
from concourse import bacc
from concourse.bass_utils import run_bass_kernel_spmd

F32 = mybir.dt.float32
BF16 = mybir.dt.bfloat16
AFT = mybir.ActivationFunctionType

NCORES = 8
B_, N, C = 1024, 68, 768
H, HD = 12, 64
N_VTS = 4
KT = C // 128              # 6 contraction tiles of 128
BL = B_ // NCORES          # 128 windows per core
GMAX = 7                   # windows per group; 7*68=476 fp32 fits a PSUM bank
GROUPS = [(ws, min(GMAX, BL - ws)) for ws in range(0, BL, GMAX)]
FDMAX = GMAX * N           # 476

_CACHE = {}


def _build_nc():
    nc = bacc.Bacc(trn_type="TRN2", target_bir_lowering=False, debug=False)

    xt_d = nc.dram_tensor("xt", [128, BL * KT * N], BF16, kind="ExternalInput")
    w1_d = nc.dram_tensor("w1", [128, 12, KT, 128], BF16, kind="ExternalInput")
    w2_d = nc.dram_tensor("w2", [128, KT, KT, 128], BF16, kind="ExternalInput")
    wp_d = nc.dram_tensor("wp", [128, KT, KT, 128], BF16, kind="ExternalInput")
    bqk_d = nc.dram_tensor("bqk", [128, 12], F32, kind="ExternalInput")
    bqkb_d = nc.dram_tensor("bqkb", [128, 12, FDMAX], BF16, kind="ExternalInput")
    bpp_d = nc.dram_tensor("bpp", [128, KT], F32, kind="ExternalInput")
    ones_d = nc.dram_tensor("ones", [N, 64], BF16, kind="ExternalInput")
    idt_d = nc.dram_tensor("idt", [128, 128], BF16, kind="ExternalInput")
    out_d = nc.dram_tensor("outt", [128, BL, KT, N], F32, kind="ExternalOutput")

    with tile.TileContext(nc) as tc:
        with (
            tc.tile_pool(name="wgt", bufs=1) as wp_pool,
            tc.tile_pool(name="xp", bufs=3) as xp,
            tc.tile_pool(name="qkp", bufs=2) as qkp,
            tc.tile_pool(name="vtp", bufs=2) as vtp,
            # all G windows' V/ES/R tiles live at once inside a group; the
            # extra bufs decouple the next group's first writes (a tighter
            # pool would make e.g. copy(V_{w}, g+1) wait on AV(w, g) via slot
            # reuse, which cycles with the shared trpa PSUM slots)
            tc.tile_pool(name="vp", bufs=GMAX + 2) as vp,
            tc.tile_pool(name="esp", bufs=GMAX + 2) as esp,
            tc.tile_pool(name="rbp", bufs=GMAX + 2) as rbp,
            tc.tile_pool(name="atp", bufs=2) as atp,
            # deep enough that proj drains never wait on an out-store DMA
            # completing (~2us each); 3 bufs measurably stalled the PE
            tc.tile_pool(name="otp", bufs=8) as otp,
            tc.tile_pool(name="pbig", bufs=2, space="PSUM") as pbig,
            tc.tile_pool(name="psc", bufs=2, space="PSUM") as psc,
            tc.tile_pool(name="prs", bufs=2, space="PSUM") as prs,
            # transpose outputs and AV accumulators share one 2-slot pool:
            # their lifetimes alternate, which double-buffers both
            tc.tile_pool(name="ptp", bufs=2, space="PSUM") as ptp,
        ):
            W1s = wp_pool.tile([128, 12, KT, 128], BF16)
            W2s = wp_pool.tile([128, KT, KT, 128], BF16)
            WPs = wp_pool.tile([128, KT, KT, 128], BF16)
            BQKs = wp_pool.tile([128, 12], F32)
            BQKB = wp_pool.tile([128, 12, FDMAX], BF16)
            BPPs = wp_pool.tile([128, KT], F32)
            ONES = wp_pool.tile([N, 64], BF16)
            IDT = wp_pool.tile([128, 128], BF16)
            def load_xt(gi):
                ws, gw = GROUPS[gi]
                # k-major group slab: the QK-gen/V^T moving operand
                # XT[:, k, :, :] is then one contiguous 476-element run
                t = xp.tile(
                    [128, KT, GMAX, N], BF16, tag="xt", name="xtile")[:, :, 0:gw]
                off = ws * KT * N
                nc.sync.dma_start(t[:], xt_d.ap()[:, off:off + gw * KT * N])
                return t

            # load order matters at startup: the first group's x slab and
            # first W1 tiles go first so QK-gen(g0) starts ~3us in instead of
            # waiting behind 5.6MB of weights (measured 25us stall); each
            # W1 j-tile lands faster than a QK-gen j-group consumes it
            XT0 = load_xt(0)
            nc.sync.dma_start(W1s[:, 0, :, :], w1_d.ap()[:, 0, :, :])
            nc.sync.dma_start(W1s[:, 1, :, :], w1_d.ap()[:, 1, :, :])
            nc.sync.dma_start(BQKs[:], bqk_d.ap())
            nc.sync.dma_start(BQKB[:], bqkb_d.ap())
            for j in range(2, 12):
                nc.sync.dma_start(W1s[:, j, :, :], w1_d.ap()[:, j, :, :])
            for ot in range(KT):
                nc.sync.dma_start(W2s[:, ot, :, :], w2_d.ap()[:, ot, :, :])
            nc.sync.dma_start(ONES[:], ones_d.ap())
            nc.sync.dma_start(IDT[:], idt_d.ap())
            for j in range(KT):
                nc.sync.dma_start(WPs[:, j, :, :], wp_d.ap()[:, j, :, :])
            nc.sync.dma_start(BPPs[:], bpp_d.ap())

            pending_proj = None

            def emit_proj(ws, gw, AT):
                fd = gw * N
                for j in range(KT):
                    po = pbig.tile([128, FDMAX], F32, tag="big", name="pbig")[:, 0:fd]
                    for kt in range(KT):
                        nc.tensor.matmul(
                            po[:], WPs[:, j, kt, :], AT[:, kt, :, :],
                            start=(kt == 0), stop=(kt == KT - 1),
                        )
                    OT = otp.tile([128, GMAX, N], F32, tag="ot", name="ot")[:, 0:gw]
                    nc.scalar.activation(
                        OT[:], po.rearrange("p (a b) -> p a b", a=gw),
                        AFT.Identity, bias=BPPs[:, j:j + 1],
                    )
                    nc.gpsimd.dma_start(out_d.ap()[:, ws:ws + gw, j, :], OT[:])

            for gi, (ws, gw) in enumerate(GROUPS):
                fd = gw * N
                XT = XT0 if gi == 0 else load_xt(gi)

                # ---- q,k in transposed layout: QKT[p, j, w, t] (j<6: q, j>=6: k)
                QKT = qkp.tile([128, 12, GMAX, N], BF16, tag="qkt", name="qkt")[:, :, 0:gw]
                for j in range(12):
                    pq = pbig.tile([128, FDMAX], F32, tag="big", name="pbig")[:, 0:fd]
                    for k in range(KT):
                        nc.tensor.matmul(
                            pq[:], W1s[:, j, k, :], XT[:, k, :, :],
                            start=(k == 0), stop=(k == KT - 1),
                        )
                    qsrc = pq.rearrange("p (a b) -> p a b", a=gw)
                    if j % 2 == 0:
                        nc.scalar.activation(
                            QKT[:, j, :, :], qsrc, AFT.Identity,
                            bias=BQKs[:, j:j + 1],
                        )
                    else:
                        qbias = BQKB.rearrange(
                            "p j (a b) -> p j a b", a=GMAX)[:, j, 0:gw, :]
                        nc.vector.tensor_add(QKT[:, j, :, :], qsrc, qbias)

                # previous group's proj, emitted here so its wait on the
                # last AV-drain multiply hides under this group's QK-gen
                if pending_proj is not None:
                    emit_proj(*pending_proj)
                    pending_proj = None

                # ---- attention scores + exp, per window
                # ES slot s = 6*half + hh holds head h = 2*hh + half, so
                # each PSUM bank sees a single PE row-group (HW hangs on
                # mixed-row-group matmuls into one bank).
                ES = {}
                for w in range(gw):
                    ES[w] = esp.tile([N, H, N], BF16, tag="es", name="es")
                    for half in range(2):
                        sc = psc.tile([N, 6, N], F32, tag="sc")
                        p0 = 64 * half
                        for hh in range(6):
                            nc.tensor.matmul(
                                sc[:, hh, :],
                                QKT[p0:p0 + 64, 6 + hh, w, :],
                                QKT[p0:p0 + 64, hh, w, :],
                                start=True, stop=True, skip_group_check=True,
                            )
                        nc.scalar.activation(
                            ES[w][:, 6 * half:6 * half + 6, :], sc[:], AFT.Exp
                        )

                # ---- v^T[o, ot, w, t]: weights stationary, x moving
                VT = vtp.tile([128, KT, GMAX, N], BF16, tag="vt", name="vt")[:, :, 0:gw]
                for ot in range(KT):
                    pv = pbig.tile([128, FDMAX], F32, tag="big", name="pbig")[:, 0:fd]
                    for k in range(KT):
                        nc.tensor.matmul(
                            pv[:], W2s[:, ot, k, :], XT[:, k, :, :],
                            start=(k == 0), stop=(k == KT - 1),
                        )
                    nc.vector.tensor_copy(
                        VT[:, ot, :, :], pv.rearrange("p (a b) -> p a b", a=gw))

                # ---- per window: rowsums (replicated across partitions by
                # the all-ones stationary) + reciprocal straight from PSUM;
                # V transposed to [token, channel] on the PE
                R = {}
                V = {}
                for w in range(gw):
                    rs = prs.tile([128, 512], F32, tag="rs", name="rs")
                    for half in range(2):
                        nc.tensor.matmul(
                            rs[64 * half:64 * half + 64, 0:6 * N],
                            ONES[:],
                            ES[w][:, 6 * half:6 * half + 6, :],
                            start=True, stop=True, skip_group_check=True,
                        )
                    R[w] = rbp.tile([128, 6, N], F32, tag="rb", name="rb")
                    nc.vector.reciprocal_approx_fast(R[w][:], rs[:, 0:6 * N])

                    tr = ptp.tile([N, KT, 128], BF16, tag="trpa", name="tr")
                    for ot in range(KT):
                        nc.tensor.matmul(
                            tr[:, ot, :], VT[:, ot, w, :], IDT[:],
                            is_transpose=True, skip_group_check=True,
                        )
                    V[w] = vp.tile([N, KT, 128], BF16, tag="v", name="v")
                    nc.scalar.copy(V[w][:], tr[:])

                # ---- AV into AT[o, kt, w, t], normalized during PSUM drain
                AT = atp.tile([128, KT, GMAX, N], BF16, tag="at", name="at")[:, :, 0:gw]
                for w in range(gw):
                    pa = ptp.tile([128, KT, N], F32, tag="trpa", name="pa")
                    for s in range(H):
                        h = 2 * (s % 6) + (s // 6)  # head held in ES slot s
                        nc.tensor.matmul(
                            pa[64 * (s // 6):64 * (s // 6) + 64, s % 6, :],
                            V[w][:, h // 2, 64 * (h % 2):64 * (h % 2) + 64],
                            ES[w][:, s, :],
                            start=True, stop=True, skip_group_check=True,
                        )
                    nc.vector.tensor_mul(AT[:, :, w, :], pa[:], R[w][:])

                pending_proj = (ws, gw, AT)

            # last group's proj
            emit_proj(*pending_proj)

    nc.compile()
    return nc


def _host_prep(x, pe, w_qkv, b_qkv, w_proj, b_proj):
    f = np.float32
    bf = ml_dtypes.bfloat16
    x = np.asarray(x, f)
    pe = np.asarray(pe, f)
    w_qkv = np.asarray(w_qkv, f)
    b_qkv = np.asarray(b_qkv, f)
    w_proj = np.asarray(w_proj, f)
    b_proj = np.asarray(b_proj, f)

    scale = f(HD ** -0.5)
    n_ = N - N_VTS
    strt = pe.shape[0] // 2 - n_ // 2

    # fold pe into x on the host; shard into per-group k-major slabs
    # [p, k, w, t] so the kernel's moving operand is contiguous
    xp = x.copy()
    xp[:, N_VTS:, :] += pe[strt:strt + n_]
    xc = xp.reshape(NCORES, BL, N, KT, 128)
    xt = np.empty((NCORES, 128, BL * KT * N), dtype=bf)
    pos = 0
    for ws, gw in GROUPS:
        blk = xc[:, ws:ws + gw].transpose(0, 4, 3, 1, 2)  # [nc, p, k, w, t]
        xt[:, :, pos:pos + gw * KT * N] = blk.reshape(NCORES, 128, -1).astype(bf)
        pos += gw * KT * N

    w_qk = np.concatenate([w_qkv[:C] * scale, w_qkv[C:2 * C]], axis=0)  # (1536, 768)
    W1 = np.ascontiguousarray(
        w_qk.reshape(12, 128, KT, 128).transpose(3, 0, 2, 1)).astype(bf)  # [p,j,k,o]
    W2 = np.ascontiguousarray(
        w_qkv[2 * C:].reshape(KT, 128, KT, 128).transpose(3, 0, 2, 1)
    ).astype(bf)                                                          # [p,ot,k,o]
    WP = np.ascontiguousarray(
        w_proj.reshape(KT, 128, KT, 128).transpose(3, 0, 2, 1)).astype(bf)  # [p,j,kt,o]

    b_qk = np.concatenate([b_qkv[:C] * scale, b_qkv[C:2 * C]])
    BQK = np.ascontiguousarray(b_qk.reshape(12, 128).T).astype(f)        # [p, j]
    BQKB = np.ascontiguousarray(np.broadcast_to(
        BQK[:, :, None], (128, 12, FDMAX))).astype(bf)                   # [p, j, fd]
    # b_v folds into the proj bias: softmax rows sum to 1, so
    # proj(attn @ (v + b_v)) = proj(attn @ v) + w_proj @ b_v
    bpp = b_proj + w_proj @ b_qkv[2 * C:]
    BPP = np.ascontiguousarray(bpp.reshape(KT, 128).T).astype(f)         # [p, j]
    ONES = np.ones((N, 64), bf)
    IDT = np.eye(128, dtype=bf)

    shared = {
        "w1": W1, "w2": W2, "wp": WP, "bqk": BQK, "bqkb": BQKB, "bpp": BPP,
        "ones": ONES, "idt": IDT,
    }
    return xt, shared


def kernel(x, pe, mask, w_qkv, b_qkv, w_proj, b_proj):
    del mask  # zeros by problem spec
    xt, shared = _host_prep(x, pe, w_qkv, b_qkv, w_proj, b_proj)

    if "nc" not in _CACHE:
        _CACHE["nc"] = _build_nc()
    nc = _CACHE["nc"]

    in_maps = [dict(shared, xt=xt[c]) for c in range(NCORES)]
    res = run_bass_kernel_spmd(
        nc, in_maps, core_ids=list(range(NCORES)),
        **_CACHE.get("run_kwargs", {}),
    )
    _CACHE["last_result"] = res

    # outt [core, p, w, j, t] -> (1024, 68, 768)
    outt = np.stack([res.results[c]["outt"] for c in range(NCORES)])
    out = np.ascontiguousarray(
        outt.transpose(0, 2, 4, 3, 1).reshape(B_, N, C))
    return out
